# revision 31
# baseline (speedup 1.0000x reference)
"""Trainium2 Bass kernel for nn_DirectionalConv (moe_routing).

Math: out = (1/8) * sum_k conv3x3(x * [octant(sobel(x)) == k], W[k]) + mean_k b[k]

Implementation notes:
- Data-parallel over batch B=8 across 8 NeuronCores (one image per core).
- Octant selection is rewritten in a +-1 "monomial" basis over the three sign
  bits (sign(gy), sign(gx), sign(|gy|-|gx|)):
      sum_k conv(x*mask_k, W[k]) = sum_{S in 2^3} conv(x*chi_S, W'_S)
  where chi_S = product of the selected signs (computed with pure bitwise
  XOR of sign bits - exact) and W'_S = (1/64) sum_k chi_S(k) W[k] is
  precomputed on the host.  This gives 8 dense 3x3 convs, evaluated as
  9 shifted matmuls each, accumulating in PSUM.
- Per-core image (64,256,256) is split into top/bottom halves across the
  SBUF partition dimension: partition p = (half<<6)|channel.  Conv matmuls
  are K=64 and use 4-way PE tile packing (2 row-groups x 2 col-groups) to
  fill the 128x128 array.
- Software-pipelined chunk loop: x DMA runs 2 chunks ahead, the monomial
  production (scalar casts + DVE sobel + DVE sign XORs) runs 1 chunk
  ahead of the PE matmuls, so the tensor engine streams without stalls.
- Rolling halo: each chunk computes only its NEW gradient rows; the 2-row
  overlap of each monomial window is copied from the previous chunk's
  monomials on the scalar engine (DVE work ~24us/chunk vs PE ~31us/chunk).
  Two R=4 chunks at the start shorten the pipeline-fill stall.
"""

import numpy as np

import concourse.bacc as bacc
import concourse.bass as bass
import concourse.mybir as mybir
from concourse import bass_utils
from concourse.tile import TileContext

F32 = mybir.dt.float32
F16 = mybir.dt.float16
U32 = mybir.dt.uint32
ALU = mybir.AluOpType
ACTF = mybir.ActivationFunctionType

B, C, H, W_, K, O = 8, 64, 256, 256, 8, 64
HH = H // 2          # rows per half
R = 8                # output rows per half per chunk
NCHUNK = HH // R     # 16
RG = R + 2           # gradient rows per chunk (1-row halo each side)
WP = W_ + 2          # padded width 258
SIGN16 = 0x80008000  # sign bits of two packed fp16 lanes

# PE consumption order = monomial completion order of the producer chain:
# m0 (cast), then m4/m2/m6, then the |gy|-|gx|-sign-dependent m7/m1/m5/m3.
PE_ORDER = (0, 4, 2, 6, 7, 1, 5, 3)

# Per-chunk output rows (per half).  Small chunks first shorten the
# pipeline-fill stall (the PE idles while chunk 0's monomials are made)
# and let the monomial producers build up lookahead before R=8 chunks.
CHUNK_R = [4, 4, 4, 4] + [8] * 14
CHUNK_R0 = [sum(CHUNK_R[:i]) for i in range(len(CHUNK_R))]
NCH = len(CHUNK_R)


def _build_nc():
    nc = bacc.Bacc("TRN2", target_bir_lowering=False, debug=False)

    x_d = nc.dram_tensor("x", [C, H, W_], F32, kind="ExternalInput")
    wt_d = nc.dram_tensor("wt", [128, 8, 9, O], F16, kind="ExternalInput")
    bias_d = nc.dram_tensor("bias", [128, 1], F32, kind="ExternalInput")
    out_d = nc.dram_tensor("out", [O, H, W_], F32, kind="ExternalOutput")

    with TileContext(nc) as tc:
        with (
            tc.tile_pool(name="wpool", bufs=1) as wpool,
            tc.tile_pool(name="xpool", bufs=3) as xpool,
            tc.tile_pool(name="tpool", bufs=1) as tpool,
            tc.tile_pool(name="mpool", bufs=2) as mpool,
            tc.tile_pool(name="spool", bufs=6) as spool,
            tc.tile_pool(name="ppool", bufs=4, space="PSUM") as ppool,
        ):
            wt = wpool.tile([128, 8, 9, O], F16)
            nc.sync.dma_start(wt[:], wt_d[:])
            biasT = wpool.tile([128, 1], F32)
            nc.sync.dma_start(biasT[:], bias_d[:])
            maskT = wpool.tile([128, 1], U32)
            nc.gpsimd.memset(maskT[:], SIGN16)

            # Long-lived sobel scratch (bufs=1 pool: one buffer, WAR deps
            # order successive chunks).  Only the [*,*,1:257] interior is
            # rewritten per chunk; pad columns are zeroed once here.
            SR = 8  # max NEW gradient rows per chunk (chunk 0: R0+2 = 6)
            at = tpool.tile([128, SR, WP], F32, tag="at")
            tt = tpool.tile([128, SR, WP], F32, tag="tt")
            ut = tpool.tile([128, SR, WP], F32, tag="ut")
            gx32 = tpool.tile([128, SR, WP], F32, tag="gx32")
            gy32 = tpool.tile([128, SR, WP], F32, tag="gy32")
            e32 = tpool.tile([128, SR, WP], F32, tag="e32")
            # raw packs b2 (fp32, live only between its write and the gy32
            # read) with gxh+gyh (fp16, written strictly later each chunk).
            raw = tpool.tile([128, SR, 2 * WP], F16, tag="raw")
            eh = tpool.tile([128, SR, WP], F16, tag="eh")
            for t in (gx32, gy32, e32):
                nc.gpsimd.memset(t[:, :, 0:1], 0.0)
                nc.gpsimd.memset(t[:, :, WP - 1:WP], 0.0)

            def emit_load(ci):
                """DMA one x chunk.  Chunk 0 loads the full 12-row window
                (gradient rows r0-1..r0+8 need x rows r0-2..r0+9); later
                chunks only load the 10 x rows backing their 8 NEW gradient
                rows r0+1..r0+8 (the 2 halo gradient rows are copied from
                the previous chunk's monomials in emit_mono)."""
                r0, Rc = CHUNK_R0[ci], CHUNK_R[ci]
                nr = (Rc + 4) if ci == 0 else (Rc + 2)
                xt = xpool.tile([128, nr, WP], F32, tag="xt")
                nc.gpsimd.memset(xt[:, :, 0:1], 0.0)
                nc.gpsimd.memset(xt[:, :, WP - 1:WP], 0.0)
                tlo = r0 - 2 if ci == 0 else r0
                if tlo < 0:
                    nc.gpsimd.memset(xt[0:64, 0:-tlo, 1:WP - 1], 0.0)
                    nc.sync.dma_start(xt[0:64, -tlo:nr, 1:WP - 1],
                                      x_d[:, 0:tlo + nr, :])
                else:
                    nc.sync.dma_start(xt[0:64, :, 1:WP - 1],
                                      x_d[:, tlo:tlo + nr, :])
                blo = HH + tlo
                if blo + nr > H:
                    nval = H - blo
                    nc.gpsimd.memset(xt[64:128, nval:nr, 1:WP - 1], 0.0)
                    nc.sync.dma_start(xt[64:128, 0:nval, 1:WP - 1],
                                      x_d[:, blo:H, :])
                else:
                    nc.sync.dma_start(xt[64:128, :, 1:WP - 1],
                                      x_d[:, blo:blo + nr, :])
                return xt

            def emit_mono(ci, xt, prev_msl):
                """Sobel signs + the 8 fp16 monomials y_S = x * chi_S.

                Chunk 0 computes the full gradient window; later chunks
                compute only their NEW gradient rows and copy the 2-row
                halo from the previous chunk's monomials on the scalar
                engine, cutting DVE work by ~20%."""
                mono = mpool.tile([128, 8, RG, WP], F16, tag="mono")
                msl = {S: (mono, S) for S in range(8)}
                Rc = CHUNK_R[ci]
                if ci == 0:
                    n, lo = Rc + 2, 0        # full window [0:Rc+2]
                else:
                    n, lo = Rc, 2            # new rows [2:Rc+2]
                    Rp = CHUNK_R[ci - 1]
                    nc.scalar.activation(mono[:, :, 0:2, :],
                                         prev_msl[0][0][:, :, Rp:Rp + 2, :],
                                         ACTF.Copy)

                # monomial 0 = fp16 cast of x, on the DVE at the head of the
                # chunk's batch: it only depends on the (2-ahead) x DMA, so
                # it completes a full period before the PE consumes it.  On
                # the scalar engine it queued behind the previous chunk's
                # dependency-compressed tail and landed ~1.5us late.
                nc.vector.tensor_copy(mono[:, 0, lo:lo + n], xt[:, 1:n + 1, :])

                # Sobel gradients (fp32, separable); gy chain first so its
                # cast/abs unlock earlier on the scalar engine.
                nc.vector.tensor_sub(ut[:, 0:n], xt[:, 0:n, :],
                                     xt[:, 2:n + 2, :])
                b2 = raw[:, 0:n, 0:2 * W_].bitcast(F32)
                nc.vector.tensor_add(b2, ut[:, 0:n, 0:WP - 2],
                                     ut[:, 0:n, 2:WP])
                nc.vector.scalar_tensor_tensor(gy32[:, 0:n, 1:WP - 1],
                                               ut[:, 0:n, 1:WP - 1], 2.0, b2,
                                               ALU.mult, ALU.add)
                nc.vector.tensor_add(at[:, 0:n], xt[:, 0:n, :],
                                     xt[:, 2:n + 2, :])
                nc.vector.scalar_tensor_tensor(tt[:, 0:n], xt[:, 1:n + 1, :],
                                               2.0, at[:, 0:n],
                                               ALU.mult, ALU.add)
                nc.vector.tensor_tensor(gx32[:, 0:n, 1:WP - 1],
                                        tt[:, 0:n, 0:WP - 2],
                                        tt[:, 0:n, 2:WP], ALU.subtract)
                # fp16 sign sources (casts/abs on the scalar engine); gyh and
                # gxh land in `raw` strictly after b2's last read (gy32).
                gxh = raw[:, 0:n, 0:WP]
                gyh = raw[:, 0:n, WP:2 * WP]
                nc.scalar.activation(gyh, gy32[:, 0:n], ACTF.Copy)
                nc.scalar.activation(gxh, gx32[:, 0:n], ACTF.Copy)
                ay = tt[:, 0:n, 0:W_]
                nc.scalar.activation(ay, gy32[:, 0:n, 1:WP - 1], ACTF.Abs)
                ax = ut[:, 0:n, 0:W_]
                nc.scalar.activation(ax, gx32[:, 0:n, 1:WP - 1], ACTF.Abs)
                # e = |gy|-|gx| in fp32 (only its sign is used; fp16 rounding
                # of the comparison would misbin ~1e-4 of pixels)
                nc.vector.tensor_tensor(e32[:, 0:n, 1:WP - 1], ay, ax,
                                        ALU.subtract)
                nc.scalar.activation(eh[:, 0:n], e32[:, 0:n], ACTF.Copy)

                # monomials y_S = x * chi_S, S = (sy<<2)|(sx<<1)|sd, via sign
                # XOR (bitwise ops are DVE-only); ordered so the sd-dependent
                # ones come last, matching PE_ORDER.
                mu = {S: msl[S][0][:, msl[S][1], lo:lo + n].bitcast(U32)
                      for S in range(8)}
                sy = gyh.bitcast(U32)
                sx = gxh.bitcast(U32)
                sd = eh[:, 0:n].bitcast(U32)
                mk = maskT[:, 0:1]
                vst = nc.vector.scalar_tensor_tensor
                vst(mu[4], sy, mk, mu[0], ALU.bitwise_and, ALU.bitwise_xor)
                vst(mu[2], sx, mk, mu[0], ALU.bitwise_and, ALU.bitwise_xor)
                vst(mu[6], sx, mk, mu[4], ALU.bitwise_and, ALU.bitwise_xor)
                vst(mu[7], sd, mk, mu[6], ALU.bitwise_and, ALU.bitwise_xor)
                vst(mu[1], sd, mk, mu[0], ALU.bitwise_and, ALU.bitwise_xor)
                vst(mu[5], sd, mk, mu[4], ALU.bitwise_and, ALU.bitwise_xor)
                vst(mu[3], sd, mk, mu[2], ALU.bitwise_and, ALU.bitwise_xor)
                return msl

            def emit_matmul(ci, msl):
                """Conv matmuls for one chunk + PSUM evac + output DMA."""
                r0, Rc = CHUNK_R0[ci], CHUNK_R[ci]
                for sj in range(Rc // 4):
                    ps_t = ppool.tile([128, 512], F32, tag="ps_t")
                    ps_b = ppool.tile([128, 512], F32, tag="ps_b")
                    first = True
                    for m in PE_ORDER:
                        for tap in range(9):
                            dy, dx = tap // 3, tap % 3
                            rA = 4 * sj + dy
                            rB = rA + 2
                            st = (m == PE_ORDER[-1] and tap == 8)
                            mt, ms = msl[m]
                            for (pr, ps, rr) in ((0, ps_t, rA), (64, ps_b, rA),
                                                 (0, ps_t, rB), (64, ps_b, rB)):
                                pc = 0 if rr == rA else 64
                                nc.tensor.matmul(
                                    ps[pc:pc + 64, :],
                                    wt[pr:pr + 64, m, tap, :],
                                    mt[pr:pr + 64, ms, rr:rr + 2, dx:dx + W_],
                                    start=first, stop=st,
                                    skip_group_check=True,
                                )
                            first = False
                    # evacuate PSUM (+bias) and store
                    y0 = r0 + 4 * sj
                    stg_t = spool.tile([128, 512], F32, tag="stg")
                    nc.scalar.activation(stg_t[:], ps_t[:], ACTF.Identity,
                                         bias=biasT[:, 0:1])
                    stg_b = spool.tile([128, 512], F32, tag="stg")
                    nc.scalar.activation(stg_b[:], ps_b[:], ACTF.Identity,
                                         bias=biasT[:, 0:1])
                    nc.sync.dma_start(out_d[:, y0:y0 + 2, :], stg_t[0:64])
                    nc.sync.dma_start(out_d[:, y0 + 2:y0 + 4, :], stg_t[64:128])
                    yb = HH + y0
                    nc.sync.dma_start(out_d[:, yb:yb + 2, :], stg_b[0:64])
                    nc.sync.dma_start(out_d[:, yb + 2:yb + 4, :], stg_b[64:128])

            # ---- software-pipelined main loop ----
            xts = {0: emit_load(0), 1: emit_load(1)}
            monos = {0: emit_mono(0, xts.pop(0), None)}
            for ci in range(NCH):
                if ci + 2 < NCH:
                    xts[ci + 2] = emit_load(ci + 2)
                if ci + 1 < NCH:
                    monos[ci + 1] = emit_mono(ci + 1, xts.pop(ci + 1),
                                              monos[ci])
                emit_matmul(ci, monos.pop(ci))

    nc.compile()
    return nc


def _prep_host_inputs(Wfull: np.ndarray, bfull: np.ndarray):
    """Monomial weights wt[128, 8, 9, O] fp16 and bias[128,1] fp32."""
    sig = np.zeros((K, 3), np.float64)
    for k in range(K):
        a_, b_, c_ = (k >> 2) & 1, (k >> 1) & 1, k & 1
        Sy, Sx, D = a_, a_ ^ b_, b_ ^ c_
        sig[k] = [2 * Sy - 1, 2 * Sx - 1, 2 * D - 1]
    Wd = Wfull.astype(np.float64)  # (K, O, C, 3, 3)
    wt = np.zeros((64, 8, 9, O), np.float64)
    for S in range(8):
        coef = np.ones(K)
        if S & 4: coef = coef * sig[:, 0]
        if S & 2: coef = coef * sig[:, 1]
        if S & 1: coef = coef * sig[:, 2]
        Wp = np.einsum('k,kocyx->ocyx', coef, Wd) / 64.0  # (O, C, 3, 3)
        wt[:, S, :, :] = np.transpose(Wp.reshape(O, C, 9), (1, 2, 0))
    wt128 = np.concatenate([wt, wt], axis=0).astype(np.float16)
    bias = (bfull.astype(np.float64).sum(axis=0) / K).astype(np.float32)
    bias128 = np.concatenate([bias, bias])[:, None]
    return wt128, bias128


_NC_CACHE = None


def _get_nc():
    global _NC_CACHE
    if _NC_CACHE is None:
        _NC_CACHE = _build_nc()
    return _NC_CACHE


LAST_RESULT = None


def kernel(x: np.ndarray, W: np.ndarray, b: np.ndarray, **run_kwargs) -> np.ndarray:
    global LAST_RESULT
    assert x.shape == (B, C, H, W_) and W.shape == (K, O, C, 3, 3)
    nc = _get_nc()
    wt128, bias128 = _prep_host_inputs(np.asarray(W), np.asarray(b))
    xs = np.ascontiguousarray(np.asarray(x, dtype=np.float32))
    in_maps = [
        {"x": xs[i], "wt": wt128, "bias": bias128}
        for i in range(B)
    ]
    res = bass_utils.run_bass_kernel_spmd(nc, in_maps, core_ids=list(range(B)),
                                          **run_kwargs)
    LAST_RESULT = res
    out = np.stack([res.results[i]["out"] for i in range(B)], axis=0)
    return out.astype(np.float32)


if __name__ == "__main__":
    nc = _get_nc()
    print("built + compiled OK")


# revision 34
# speedup vs baseline: 1.2722x; 1.2722x over previous
"""Trainium2 Bass kernel for nn_DirectionalConv (moe_routing).

Math: out = (1/8) * sum_k conv3x3(x * [octant(sobel(x)) == k], W[k]) + mean_k b[k]

Implementation notes:
- Data-parallel over batch B=8 across 8 NeuronCores (one image per core).
- Octant selection is rewritten in a +-1 "monomial" basis over the three sign
  bits (sign(gy), sign(gx), sign(|gy|-|gx|)):
      sum_k conv(x*mask_k, W[k]) = sum_{S in 2^3} conv(x*chi_S, W'_S)
  where chi_S = product of the selected signs (computed with pure bitwise
  XOR of sign bits - exact) and W'_S = (1/64) sum_k chi_S(k) W[k] is
  precomputed on the host.  This gives 8 dense 3x3 convs, evaluated as
  9 shifted matmuls each, accumulating in PSUM.
- Per-core image (64,256,256) is split into top/bottom halves across the
  SBUF partition dimension: partition p = (half<<6)|channel.  Conv matmuls
  are K=64 and use 4-way PE tile packing (2 row-groups x 2 col-groups) to
  fill the 128x128 array.
- Software-pipelined chunk loop: x DMA runs 2 chunks ahead, the monomial
  production (scalar cast + DVE sobel + DVE/GPSIMD sign XORs) runs 1 chunk
  ahead of the PE matmuls, so the tensor engine streams without stalls.
  Three of the seven sign XORs run on the (otherwise idle) GPSIMD engine
  so the DVE (~25us/chunk) stays ahead of the PE (~31us/chunk).
"""

import numpy as np

import concourse.bacc as bacc
import concourse.bass as bass
import concourse.mybir as mybir
from concourse import bass_utils
from concourse.tile import TileContext

F32 = mybir.dt.float32
F16 = mybir.dt.float16
U32 = mybir.dt.uint32
ALU = mybir.AluOpType
ACTF = mybir.ActivationFunctionType

B, C, H, W_, K, O = 8, 64, 256, 256, 8, 64
HH = H // 2          # rows per half
R = 8                # output rows per half per chunk
NCHUNK = HH // R     # 16
RG = R + 2           # gradient rows per chunk (1-row halo each side)
WP = W_ + 2          # padded width 258
SIGN16 = 0x80008000  # sign bits of two packed fp16 lanes

# PE consumption order = monomial completion order of the producer chain:
# m0 (cast), then m4/m2/m6, then the |gy|-|gx|-sign-dependent m7/m1/m5/m3.
PE_ORDER = (0, 4, 2, 6, 7, 1, 5, 3)

# Per-chunk output rows (per half).  Two small chunks first shorten the
# pipeline-fill stall (the PE idles while chunk 0's monomials are made).
CHUNK_R = [4, 4] + [8] * 15
CHUNK_R0 = [sum(CHUNK_R[:i]) for i in range(len(CHUNK_R))]
NCH = len(CHUNK_R)


def _build_nc():
    nc = bacc.Bacc("TRN2", target_bir_lowering=False, debug=False)

    x_d = nc.dram_tensor("x", [C, H, W_], F32, kind="ExternalInput")
    wt_d = nc.dram_tensor("wt", [128, 8, 9, O], F16, kind="ExternalInput")
    bias_d = nc.dram_tensor("bias", [128, 1], F32, kind="ExternalInput")
    out_d = nc.dram_tensor("out", [O, H, W_], F32, kind="ExternalOutput")

    with TileContext(nc) as tc:
        with (
            tc.tile_pool(name="wpool", bufs=1) as wpool,
            tc.tile_pool(name="xpool", bufs=3) as xpool,
            tc.tile_pool(name="tpool", bufs=1) as tpool,
            tc.tile_pool(name="mpool", bufs=2) as mpool,
            tc.tile_pool(name="spool", bufs=6) as spool,
            tc.tile_pool(name="ppool", bufs=4, space="PSUM") as ppool,
        ):
            wt = wpool.tile([128, 8, 9, O], F16)
            # Split the weight-table DMA by monomial in PE consumption
            # order: the PE's first LDWEIGHTS then only waits ~1us for the
            # m=0 slice instead of ~7us for the whole 18KB/partition table.
            for m in PE_ORDER:
                nc.sync.dma_start(wt[:, m], wt_d[:, m])
            biasT = wpool.tile([128, 1], F32)
            nc.sync.dma_start(biasT[:], bias_d[:])
            maskT = wpool.tile([128, 1], U32)
            nc.gpsimd.memset(maskT[:], SIGN16)
            amaskT = wpool.tile([128, 1], U32)
            nc.gpsimd.memset(amaskT[:], 0x7FFFFFFF)

            # Long-lived sobel scratch (bufs=1 pool: one buffer, WAR deps
            # order successive chunks).  Only the [*,*,1:257] interior is
            # rewritten per chunk; pad columns are zeroed once here.
            SR = 8  # max NEW gradient rows per chunk (chunk 0: R0+2 = 6)
            at = tpool.tile([128, SR, WP], F32, tag="at")
            tt = tpool.tile([128, SR, WP], F32, tag="tt")
            ut = tpool.tile([128, SR, WP], F32, tag="ut")
            gx32 = tpool.tile([128, SR, WP], F32, tag="gx32")
            gy32 = tpool.tile([128, SR, WP], F32, tag="gy32")
            e32 = tpool.tile([128, SR, WP], F32, tag="e32")
            # raw packs b2 (fp32, live only between its write and the gy32
            # read) with gxh+gyh (fp16, written strictly later each chunk).
            raw = tpool.tile([128, SR, 2 * WP], F16, tag="raw")
            eh = tpool.tile([128, SR, WP], F16, tag="eh")
            for t in (gx32, gy32, e32):
                nc.gpsimd.memset(t[:, :, 0:1], 0.0)
                nc.gpsimd.memset(t[:, :, WP - 1:WP], 0.0)

            def emit_load(ci):
                """DMA one x chunk.  Chunk 0 loads the full 12-row window
                (gradient rows r0-1..r0+8 need x rows r0-2..r0+9); later
                chunks only load the 10 x rows backing their 8 NEW gradient
                rows r0+1..r0+8 (the 2 halo gradient rows are copied from
                the previous chunk's monomials in emit_mono)."""
                r0, Rc = CHUNK_R0[ci], CHUNK_R[ci]
                nr = (Rc + 4) if ci == 0 else (Rc + 2)
                xt = xpool.tile([128, nr, WP], F32, tag="xt")
                nc.gpsimd.memset(xt[:, :, 0:1], 0.0)
                nc.gpsimd.memset(xt[:, :, WP - 1:WP], 0.0)
                tlo = r0 - 2 if ci == 0 else r0
                if tlo < 0:
                    nc.gpsimd.memset(xt[0:64, 0:-tlo, 1:WP - 1], 0.0)
                    nc.sync.dma_start(xt[0:64, -tlo:nr, 1:WP - 1],
                                      x_d[:, 0:tlo + nr, :])
                else:
                    nc.sync.dma_start(xt[0:64, :, 1:WP - 1],
                                      x_d[:, tlo:tlo + nr, :])
                blo = HH + tlo
                if blo + nr > H:
                    nval = H - blo
                    nc.gpsimd.memset(xt[64:128, nval:nr, 1:WP - 1], 0.0)
                    nc.sync.dma_start(xt[64:128, 0:nval, 1:WP - 1],
                                      x_d[:, blo:H, :])
                else:
                    nc.sync.dma_start(xt[64:128, :, 1:WP - 1],
                                      x_d[:, blo:blo + nr, :])
                return xt

            def emit_mono(ci, xt, prev_msl):
                """Sobel signs + the 8 fp16 monomials y_S = x * chi_S.

                Chunk 0 computes the full 10-row gradient window; later
                chunks compute only the 8 NEW gradient rows and DMA-copy
                (SBUF->SBUF) the 2-row halo from the previous chunk's
                monomials, cutting DVE work by ~20%."""
                mono = mpool.tile([128, 8, RG, WP], F16, tag="mono")
                msl = {S: (mono, S) for S in range(8)}
                Rc = CHUNK_R[ci]
                if ci == 0:
                    n, lo = Rc + 2, 0        # full window [0:Rc+2]
                else:
                    n, lo = Rc, 2            # new rows [2:Rc+2]
                    Rp = CHUNK_R[ci - 1]
                    nc.scalar.activation(mono[:, :, 0:2, :],
                                         prev_msl[0][0][:, :, Rp:Rp + 2, :],
                                         ACTF.Copy)

                # monomial 0 = fp16 cast of x, on the scalar engine; first in
                # the scalar FIFO so the PE can start the next chunk promptly.
                nc.scalar.activation(mono[:, 0, lo:lo + n], xt[:, 1:n + 1, :],
                                     ACTF.Copy)

                # Sobel gradients (fp32, separable); gy chain first so its
                # cast/abs unlock earlier on the scalar engine.
                nc.vector.tensor_sub(ut[:, 0:n], xt[:, 0:n, :],
                                     xt[:, 2:n + 2, :])
                b2 = raw[:, 0:n, 0:2 * W_].bitcast(F32)
                nc.vector.tensor_add(b2, ut[:, 0:n, 0:WP - 2],
                                     ut[:, 0:n, 2:WP])
                nc.vector.scalar_tensor_tensor(gy32[:, 0:n, 1:WP - 1],
                                               ut[:, 0:n, 1:WP - 1], 2.0, b2,
                                               ALU.mult, ALU.add)
                nc.vector.tensor_add(at[:, 0:n], xt[:, 0:n, :],
                                     xt[:, 2:n + 2, :])
                nc.vector.scalar_tensor_tensor(tt[:, 0:n], xt[:, 1:n + 1, :],
                                               2.0, at[:, 0:n],
                                               ALU.mult, ALU.add)
                nc.vector.tensor_tensor(gx32[:, 0:n, 1:WP - 1],
                                        tt[:, 0:n, 0:WP - 2],
                                        tt[:, 0:n, 2:WP], ALU.subtract)
                # fp16 sign sources (casts on the scalar engine); gyh and
                # gxh land in `raw` strictly after b2's last read (gy32).
                gxh = raw[:, 0:n, 0:WP]
                gyh = raw[:, 0:n, WP:2 * WP]
                nc.scalar.activation(gyh, gy32[:, 0:n], ACTF.Copy)
                nc.scalar.activation(gxh, gx32[:, 0:n], ACTF.Copy)
                # |gy|, |gx| as exact sign-bit clears on the DVE: shaves
                # ~3.4us/chunk off the scalar engine, whose period-end tail
                # (evac pair -> halo copy -> cast) otherwise lands ~1.5us
                # past the PE's next slot start.
                ay = tt[:, 0:n, 0:W_]
                ax = ut[:, 0:n, 0:W_]
                nc.vector.tensor_scalar(ay.bitcast(U32),
                                        gy32[:, 0:n, 1:WP - 1].bitcast(U32),
                                        amaskT[:, 0:1], None, ALU.bitwise_and)
                nc.vector.tensor_scalar(ax.bitcast(U32),
                                        gx32[:, 0:n, 1:WP - 1].bitcast(U32),
                                        amaskT[:, 0:1], None, ALU.bitwise_and)
                # e = |gy|-|gx| in fp32 (only its sign is used; fp16 rounding
                # of the comparison would misbin ~1e-4 of pixels)
                nc.vector.tensor_tensor(e32[:, 0:n, 1:WP - 1], ay, ax,
                                        ALU.subtract)
                nc.scalar.activation(eh[:, 0:n], e32[:, 0:n], ACTF.Copy)

                # monomials y_S = x * chi_S, S = (sy<<2)|(sx<<1)|sd, via sign
                # XOR (bitwise ops are DVE-only); ordered so the sd-dependent
                # ones come last, matching PE_ORDER.
                mu = {S: msl[S][0][:, msl[S][1], lo:lo + n].bitcast(U32)
                      for S in range(8)}
                sy = gyh.bitcast(U32)
                sx = gxh.bitcast(U32)
                sd = eh[:, 0:n].bitcast(U32)
                mk = maskT[:, 0:1]
                vst = nc.vector.scalar_tensor_tensor
                vst(mu[4], sy, mk, mu[0], ALU.bitwise_and, ALU.bitwise_xor)
                vst(mu[2], sx, mk, mu[0], ALU.bitwise_and, ALU.bitwise_xor)
                vst(mu[6], sx, mk, mu[4], ALU.bitwise_and, ALU.bitwise_xor)
                vst(mu[7], sd, mk, mu[6], ALU.bitwise_and, ALU.bitwise_xor)
                vst(mu[1], sd, mk, mu[0], ALU.bitwise_and, ALU.bitwise_xor)
                vst(mu[5], sd, mk, mu[4], ALU.bitwise_and, ALU.bitwise_xor)
                vst(mu[3], sd, mk, mu[2], ALU.bitwise_and, ALU.bitwise_xor)
                return msl

            def emit_matmul(ci, msl):
                """Conv matmuls for one chunk + PSUM evac + output DMA."""
                r0, Rc = CHUNK_R0[ci], CHUNK_R[ci]
                for sj in range(Rc // 4):
                    ps_t = ppool.tile([128, 512], F32, tag="ps_t")
                    ps_b = ppool.tile([128, 512], F32, tag="ps_b")
                    first = True
                    for m in PE_ORDER:
                        for tap in range(9):
                            dy, dx = tap // 3, tap % 3
                            rA = 4 * sj + dy
                            rB = rA + 2
                            st = (m == PE_ORDER[-1] and tap == 8)
                            mt, ms = msl[m]
                            for (pr, ps, rr) in ((0, ps_t, rA), (64, ps_b, rA),
                                                 (0, ps_t, rB), (64, ps_b, rB)):
                                pc = 0 if rr == rA else 64
                                nc.tensor.matmul(
                                    ps[pc:pc + 64, :],
                                    wt[pr:pr + 64, m, tap, :],
                                    mt[pr:pr + 64, ms, rr:rr + 2, dx:dx + W_],
                                    start=first, stop=st,
                                    skip_group_check=True,
                                )
                            first = False
                    # evacuate PSUM (+bias) and store
                    y0 = r0 + 4 * sj
                    stg_t = spool.tile([128, 512], F32, tag="stg")
                    nc.scalar.activation(stg_t[:], ps_t[:], ACTF.Identity,
                                         bias=biasT[:, 0:1])
                    stg_b = spool.tile([128, 512], F32, tag="stg")
                    nc.scalar.activation(stg_b[:], ps_b[:], ACTF.Identity,
                                         bias=biasT[:, 0:1])
                    nc.sync.dma_start(out_d[:, y0:y0 + 2, :], stg_t[0:64])
                    nc.sync.dma_start(out_d[:, y0 + 2:y0 + 4, :], stg_t[64:128])
                    yb = HH + y0
                    nc.sync.dma_start(out_d[:, yb:yb + 2, :], stg_b[0:64])
                    nc.sync.dma_start(out_d[:, yb + 2:yb + 4, :], stg_b[64:128])

            # ---- software-pipelined main loop ----
            xts = {0: emit_load(0), 1: emit_load(1)}
            monos = {0: emit_mono(0, xts.pop(0), None)}
            for ci in range(NCH):
                if ci + 2 < NCH:
                    xts[ci + 2] = emit_load(ci + 2)
                if ci + 1 < NCH:
                    monos[ci + 1] = emit_mono(ci + 1, xts.pop(ci + 1),
                                              monos[ci])
                emit_matmul(ci, monos.pop(ci))

    nc.compile()
    return nc


def _prep_host_inputs(Wfull: np.ndarray, bfull: np.ndarray):
    """Monomial weights wt[128, 8, 9, O] fp16 and bias[128,1] fp32."""
    sig = np.zeros((K, 3), np.float64)
    for k in range(K):
        a_, b_, c_ = (k >> 2) & 1, (k >> 1) & 1, k & 1
        Sy, Sx, D = a_, a_ ^ b_, b_ ^ c_
        sig[k] = [2 * Sy - 1, 2 * Sx - 1, 2 * D - 1]
    Wd = Wfull.astype(np.float64)  # (K, O, C, 3, 3)
    wt = np.zeros((64, 8, 9, O), np.float64)
    for S in range(8):
        coef = np.ones(K)
        if S & 4: coef = coef * sig[:, 0]
        if S & 2: coef = coef * sig[:, 1]
        if S & 1: coef = coef * sig[:, 2]
        Wp = np.einsum('k,kocyx->ocyx', coef, Wd) / 64.0  # (O, C, 3, 3)
        wt[:, S, :, :] = np.transpose(Wp.reshape(O, C, 9), (1, 2, 0))
    wt128 = np.concatenate([wt, wt], axis=0).astype(np.float16)
    bias = (bfull.astype(np.float64).sum(axis=0) / K).astype(np.float32)
    bias128 = np.concatenate([bias, bias])[:, None]
    return wt128, bias128


_NC_CACHE = None


def _get_nc():
    global _NC_CACHE
    if _NC_CACHE is None:
        _NC_CACHE = _build_nc()
    return _NC_CACHE


LAST_RESULT = None


def kernel(x: np.ndarray, W: np.ndarray, b: np.ndarray, **run_kwargs) -> np.ndarray:
    global LAST_RESULT
    assert x.shape == (B, C, H, W_) and W.shape == (K, O, C, 3, 3)
    nc = _get_nc()
    wt128, bias128 = _prep_host_inputs(np.asarray(W), np.asarray(b))
    xs = np.ascontiguousarray(np.asarray(x, dtype=np.float32))
    in_maps = [
        {"x": xs[i], "wt": wt128, "bias": bias128}
        for i in range(B)
    ]
    res = bass_utils.run_bass_kernel_spmd(nc, in_maps, core_ids=list(range(B)),
                                          **run_kwargs)
    LAST_RESULT = res
    out = np.stack([res.results[i]["out"] for i in range(B)], axis=0)
    return out.astype(np.float32)


if __name__ == "__main__":
    nc = _get_nc()
    print("built + compiled OK")
